# revision 15
# baseline (speedup 1.0000x reference)
"""Trainium2 Bass kernel for a dense CNN (conv trunk + SPP + 3 FC layers).

Sharding over 8 NeuronCores:
  - Conv trunk data-parallel over batch (8 images/core). Activations live in
    SBUF in a "strip" layout [C, H, B*W] (batch folded into width), so every
    conv matmul's moving operand is a flat contiguous slice with N in
    [256, 512].
  - conv1 (7x7 s2 p3) is phase-decomposed on host: 2x2 input phases turn it
    into 16 taps of a 4x4 s1 conv over phase images. 8 taps are stacked into
    the contraction dim via a host-built replicated strip; the other 8 reuse
    the same strip at a +2 column offset (second accumulating matmul).
  - conv1 writes row-triples into 3-bank PSUM tiles; ONE DVE XY-max-reduce
    per half-row fuses the evict with the whole 3x3/s3 pool1 (bias+relu
    applied after pooling - max commutes with monotone relu(x+b)).
  - conv trunk activations/weights bf16; evicts on Act; pool2/SPP on DVE.
  - FC stage tensor-parallel: AllGather(feats, split per channel-half) ->
    fc1 (512 outs/core) -> PE transpose -> fc2 partials -> AllReduce in two
    pipelined halves -> relu -> fc3 125 outs/core, host concatenates.
"""

import sys

sys.path.insert(0, "/opt/trn_rl_repo")

import numpy as np
import ml_dtypes

import concourse.mybir as mybir
import concourse.tile as tile
from concourse import bacc
from concourse.bass_utils import run_bass_kernel_spmd

F32 = mybir.dt.float32
F32R = mybir.dt.float32r
BF16 = mybir.dt.bfloat16
RELU = mybir.ActivationFunctionType.Relu
COPY = mybir.ActivationFunctionType.Copy
MAX = mybir.AluOpType.max
ADD = mybir.AluOpType.add
AXX = mybir.AxisListType.X
AXY = mybir.AxisListType.XY

N_CORES = 8
B = 64
BL = B // N_CORES
W1S = 922        # conv1 strip row width (8*115 + 2 pad)
C1BAND = 9       # conv1 band rows (multiple of 3)
NF3 = 1000 // N_CORES  # fc3 outputs per core
SPP_LEVELS = (6, 3, 2, 1)


def _spp_bins():
    bins = []
    for L in SPP_LEVELS:
        bd = [((i * 10) // L, -((-(i + 1) * 10) // L)) for i in range(L)]
        for i0, i1 in bd:
            for j0, j1 in bd:
                bins.append((i0, i1, j0, j1))
    return bins


# ----------------------------------------------------------------------------
# device program
# ----------------------------------------------------------------------------

def build_program():
    nc = bacc.Bacc(None, target_bir_lowering=False)

    def din(name, shape, dt):
        return nc.dram_tensor(name, list(shape), dt, kind="ExternalInput")

    c1rep = din("c1rep", [96, 111, W1S], BF16)
    w1g = din("w1g", [2, 96, 64], BF16)
    b1 = din("b1", [64, 1], F32)
    w2s = [din(f"w2_{i}", [2, 2, 64 if i == 0 else 128, 128], BF16) for i in range(4)]
    b2s = [din(f"b2_{i}", [128, 1], F32) for i in range(4)]
    w3s = [din(f"w3_{i}", [2, 2, 128 if i == 0 else 256, 256], BF16) for i in range(6)]
    b3s = [din(f"b3_{i}", [128, 2], F32) for i in range(6)]
    w1c = din("w1c", [100, 128, 512], BF16)
    b1c = din("b1c", [1, 512], BF16)
    w2c = din("w2c", [512, 4096], BF16)
    b2f = din("b2f", [1, 4096], BF16)
    ones0_in = din("ones0", [1, B], BF16)
    w3T = din("w3T", [4096, NF3], BF16)
    b3f = din("b3f", [1, NF3], BF16)

    out = nc.dram_tensor("out", [64, NF3], F32, kind="ExternalOutput")

    ag_src = nc.dram_tensor("ag_src", [2, 128, BL, 50], BF16)
    ag_dst = nc.dram_tensor("ag_dst", [2, N_CORES, 128, BL, 50], BF16,
                            addr_space="Shared")
    ar_src = nc.dram_tensor("ar_src", [4096, B], BF16)
    ar_dst = nc.dram_tensor("ar_dst", [4096, B], BF16, addr_space="Shared")
    warm_dst = nc.dram_tensor("warm_dst", [N_CORES, 128, BL, 50], BF16,
                              addr_space="Shared")
    warm2_dst = nc.dram_tensor("warm2_dst", [2048, B], BF16, addr_space="Shared")

    tc_cm = tile.TileContext(nc)
    tc = tc_cm.__enter__()

    const_cm = tc.tile_pool(name="const", bufs=1); const = const_cm.__enter__()
    psum_holder = {}

    def ps(name):
        return psum_holder["pool"].tile([128, 512], F32, name=name, tag="ps")

    # --- small resident constants -------------------------------------------
    w1sb = const.tile([96, 2, 64], BF16, name="w1sb")
    nc.sync.dma_start(w1sb[:], w1g[:].transpose((1, 0, 2)))
    b1sb = const.tile([64, 1], F32, name="b1sb")
    nc.sync.dma_start(b1sb[:], b1[:])
    # ========================================================================
    # conv1 + pool1 (fused into PSUM XY-max-reduces per row-triple)
    # ========================================================================
    p1_cm = tc.tile_pool(name="p1pool", bufs=1); p1pool = p1_cm.__enter__()
    pooled1 = p1pool.tile([64, 38, 296], BF16, name="pooled1")
    nc.gpsimd.memset(pooled1[:, 37, :], 0.0)

    ps1_cm = tc.tile_pool(name="psum1", bufs=2, space="PSUM")
    psum1 = ps1_cm.__enter__()
    band_cm = tc.tile_pool(name="band", bufs=2); band_pool = band_cm.__enter__()
    r0 = 0
    while r0 < 111:
        nr = 3 if r0 < 9 else min(C1BAND, 111 - r0)
        rep = band_pool.tile([96, C1BAND, W1S], BF16, name="rep", tag="rep", bufs=4)
        step = 1 if r0 == 0 else 3
        for rr in range(0, nr, step):
            rn = min(step, nr - rr)
            nc.sync.dma_start(rep[:, rr:rr + rn, :],
                              c1rep[:, r0 + rr:r0 + rr + rn, :])
        t1b = band_pool.tile([64, C1BAND // 3, 296], BF16, name="t1b", tag="t1b")
        rep_flat = rep[:].rearrange("k h w -> k (h w)")
        for tri in range(nr // 3):
            for ci, u0 in enumerate((0, 460)):
                pp = psum1.tile([64, 3, 512], F32, name="pp", tag="tri")
                for r in range(3):
                    for wg in range(2):
                        base = (3 * tri + r) * W1S + u0 + 2 * wg
                        nc.tensor.matmul(pp[:, r, :460], w1sb[:, wg, :],
                                         rep_flat[:, base:base + 460],
                                         start=(wg == 0), stop=(wg == 1))
                # fused evict + 3x3/s3 max-pool: reduce rows (Y) and col
                # triples (X) in one pass from PSUM
                v = pp[:, :, :460].rearrange("c r (b v) -> c b v r", b=4) \
                    [:, :, :111, :].rearrange("c b (w3 t) r -> c b w3 r t", t=3)
                dv = t1b[:, tri, ci * 148:ci * 148 + 148].rearrange(
                    "c (b w3) -> c b w3", b=4)
                nc.vector.tensor_reduce(dv, v, AXY, MAX)
        pr0, prn = r0 // 3, nr // 3
        # bias + relu on the pooled rows (Act)
        nc.scalar.activation(pooled1[:, pr0:pr0 + prn, :], t1b[:, :prn, :],
                             RELU, bias=b1sb[:])
        r0 += nr
        if r0 == 3:
            # emit the late-phase constant loads after band 0 is in flight
            w2sb, b2sb = [], []
            for i in range(4):
                cin = 64 if i == 0 else 128
                t = const.tile([cin, 2, 2, 128], BF16, name=f"w2sb{i}")
                nc.sync.dma_start(t[:], w2s[i][:].transpose((2, 0, 1, 3)))
                w2sb.append(t)
                tb = const.tile([128, 1], F32, name=f"b2sb{i}")
                nc.sync.dma_start(tb[:], b2s[i][:])
                b2sb.append(tb)
            b3sb = []
            for i in range(6):
                tb = const.tile([128, 2], F32, name=f"b3sb{i}")
                nc.sync.dma_start(tb[:], b3s[i][:])
                b3sb.append(tb)
            ones_bf = const.tile([1, 64], BF16, name="ones_bf")
            nc.vector.memset(ones_bf[:], 1.0)
            b1csb = const.tile([1, 512], BF16, name="b1csb")
            nc.sync.dma_start(b1csb[:], b1c[:])
            b2fsb = const.tile([1, 4096], BF16, name="b2fsb")
            nc.sync.dma_start(b2fsb[:], b2f[:])
            ones0 = const.tile([1, B], BF16, name="ones0")
            nc.sync.dma_start(ones0[:], ones0_in[:])
            b3fsb = const.tile([1, NF3], BF16, name="b3fsb")
            nc.sync.dma_start(b3fsb[:], b3f[:])
    band_cm.__exit__(None, None, None)
    ps1_cm.__exit__(None, None, None)

    psum_cm = tc.tile_pool(name="psum", bufs=7, space="PSUM")
    psum_holder["pool"] = psum = psum_cm.__enter__()

    # ========================================================================
    # conv2 block (4 layers), strip width 37/img
    # ========================================================================
    a2_cm = tc.tile_pool(name="a2pool", bufs=2); a2pool = a2_cm.__enter__()
    cur = pooled1
    for li in range(4):
        hout = 36 - li if li < 3 else 33
        w = w2sb[li]
        cur_flat = cur[:].rearrange("c h w -> c (h w)")
        dst = a2pool.tile([128, 37, 296], BF16, name=f"a2_{li}", tag="a2")
        if li < 3:
            nc.gpsimd.memset(dst[:, hout, :], 0.0)
        for r in range(hout):
            p = ps("p_c2")
            k = 0
            for di in range(2):
                for dj in range(2):
                    base = (r + di) * 296 + dj
                    nc.tensor.matmul(p[:, :296], w[:, di, dj, :],
                                     cur_flat[:, base:base + 296],
                                     start=(k == 0), stop=(k == 3))
                    k += 1
            nc.scalar.activation(dst[:, r, :], p[:, :296], RELU,
                                 bias=b2sb[li][:])
        cur = dst

    # warm up the collectives firmware with EXACT-signature dummies
    # (plan caching is per payload shape/op; junk data, junk dsts)
    nc.gpsimd.collective_compute(
        "AllGather", mybir.AluOpType.bypass,
        replica_groups=[list(range(N_CORES))],
        ins=[ag_src[0].opt()], outs=[warm_dst[:].opt()])
    nc.gpsimd.collective_compute(
        "AllReduce", mybir.AluOpType.add,
        replica_groups=[list(range(N_CORES))],
        ins=[ar_src[0:2048].opt()], outs=[warm2_dst[:].opt()])

    # pool2: 2x2 s2 on DVE -> [128, 16, 8*16] (+ zero pad row)
    pooled2 = const.tile([128, 17, 128], BF16, name="pooled2")
    nc.gpsimd.memset(pooled2[:, 16, :], 0.0)
    u2 = a2pool.tile([128, 33, 128], BF16, name="u2", tag="u2", bufs=1)
    t2v = cur[:, :33, :].rearrange("c h (b w) -> c h b w", w=37)
    u2v = u2[:].rearrange("c h (b w) -> c h b w", w=16)
    for (h0, h1) in ((0, 16), (16, 33)):
        nc.vector.tensor_tensor(u2v[:, h0:h1], t2v[:, h0:h1, :, 0:32:2],
                                t2v[:, h0:h1, :, 1:33:2], MAX)
        p0, p1 = h0 // 2, h1 // 2
        nc.vector.tensor_tensor(pooled2[:, p0:p1, :], u2[:, 2 * p0:2 * p1:2, :],
                                u2[:, 2 * p0 + 1:2 * p1 + 1:2, :], MAX)
    a2_cm.__exit__(None, None, None)
    p1_cm.__exit__(None, None, None)

    # ========================================================================
    # conv3 block (6 layers), strip width 16/img
    # ========================================================================
    w1_cm = tc.tile_pool(name="w1pool", bufs=13); w1pool = w1_cm.__enter__()
    fg_cm = tc.tile_pool(name="fgpool", bufs=1); fgpool = fg_cm.__enter__()
    featg2 = fgpool.tile([128, 2, N_CORES, BL, 50], BF16, name="featg2")
    fe_cm = tc.tile_pool(name="fepool", bufs=1); fepool = fe_cm.__enter__()
    a3_cm = tc.tile_pool(name="a3pool", bufs=2); a3pool = a3_cm.__enter__()
    w3_cm = tc.tile_pool(name="w3pool", bufs=6); w3pool = w3_cm.__enter__()

    # all conv3 weight tiles up-front (6 bufs -> no WAR stalls in the queue)
    w3tiles = []
    for li in range(6):
        nkt = 1 if li == 0 else 2
        wsb = w3pool.tile([128, nkt, 2, 2, 256], BF16, name=f"w3sb{li}", tag="w3s")
        for kt in range(nkt):
            nc.sync.dma_start(
                wsb[:, kt, :, :, :],
                w3s[li][:, :, kt * 128:(kt + 1) * 128, :].transpose((2, 0, 1, 3)))
        w3tiles.append(wsb)

    # fc1 weight tiles, 8 K-tiles per DMA (stream during conv3 + fc1)
    w1tiles = []
    for g in range(13):
        nkt = 8 if g < 12 else 4
        wt = w1pool.tile([128, 8, 512], BF16, name="w1t", tag="w1t", bufs=13)
        nc.sync.dma_start(wt[:, :nkt, :],
                          w1c[8 * g:8 * g + nkt].transpose((1, 0, 2)))
        w1tiles.append(wt)

    a3prev = None
    for li in range(6):
        nkt = 1 if li == 0 else 2
        hin = 16 - li
        hout = hin - 1
        wsb = w3tiles[li]
        dst = a3pool.tile([128, 2, 16, 128], BF16, name=f"a3_{li}", tag="a3")
        nc.gpsimd.memset(dst[:, :, hout, :], 0.0)
        if li == 0:
            src_flat = [pooled2[:].rearrange("c h w -> c (h w)")]
        else:
            src_flat = [a3prev[:, kt, :, :].rearrange("c h w -> c (h w)")
                        for kt in range(2)]
        if li == 5:
            rcs = []
            r = 0
            while r < hout:
                rcs.append((r, min(3, hout - r)))
                r += rcs[-1][1]
            loop = [(mt, r, nr) for mt in range(2) for (r, nr) in rcs]
        else:
            loop = []
            r = 0
            while r < hout:
                nr = min(3, hout - r)
                loop += [(0, r, nr), (1, r, nr)]
                r += nr
        for (mt, r, nr) in loop:
            p = ps("p_c3")
            k = 0
            nmm = 4 * nkt
            for kt in range(nkt):
                for di in range(2):
                    for dj in range(2):
                        base = (r + di) * 128 + dj
                        nc.tensor.matmul(
                            p[:, :nr * 128],
                            wsb[:, kt, di, dj, mt * 128:(mt + 1) * 128],
                            src_flat[kt][:, base:base + nr * 128],
                            start=(k == 0), stop=(k == nmm - 1))
                        k += 1
            dv = dst[:, mt, r:r + nr, :].rearrange("c h w -> c (h w)")
            nc.scalar.activation(dv, p[:, :nr * 128], RELU,
                                 bias=b3sb[li][:, mt:mt + 1])
        a3prev = dst

    # ========================================================================
    # SPP: per-bin sums (mean folded into fc1 weights) -> feats bf16,
    # AllGather split per channel-half so ct=0 overlaps conv3 mt=1 + ct=1
    # ========================================================================
    feats = fepool.tile([128, 2, BL, 50], BF16, name="feats")
    rsum = fepool.tile([128, 2, BL, 12, 10], F32, name="rsum")
    h5 = a3prev
    with nc.allow_low_precision(reason="SPP small-window sums"):
        for ct in range(2):
            rb = 0
            rowbin_of = {}
            for L in SPP_LEVELS:
                for i in range(L):
                    i0, i1 = (i * 10) // L, -((-(i + 1) * 10) // L)
                    vv = h5[:, ct, i0:i1, :].rearrange(
                        "c h (b w) -> c b w h", w=16)[:, :, :10, :]
                    nc.vector.tensor_reduce(rsum[:, ct, :, rb, :], vv, AXX, ADD)
                    rowbin_of[(L, i)] = rb
                    rb += 1
            kbase = 0
            for L in SPP_LEVELS:
                for i in range(L):
                    rbi = rowbin_of[(L, i)]
                    for j in range(L):
                        j0, j1 = (j * 10) // L, -((-(j + 1) * 10) // L)
                        nc.vector.tensor_reduce(
                            feats[:, ct, :, kbase + i * L + j],
                            rsum[:, ct, :, rbi, j0:j1], AXX, ADD)
                kbase += L * L
            nc.sync.dma_start(ag_src[ct], feats[:, ct, :, :])
            nc.gpsimd.collective_compute(
                "AllGather", mybir.AluOpType.bypass,
                replica_groups=[list(range(N_CORES))],
                ins=[ag_src[ct].opt()], outs=[ag_dst[ct].opt()])
            nc.sync.dma_start(featg2[:, ct, :, :, :],
                              ag_dst[ct].transpose((1, 0, 2, 3)))
    w3_cm.__exit__(None, None, None)
    a3_cm.__exit__(None, None, None)

    fe_cm.__exit__(None, None, None)
    fc_cm = tc.tile_pool(name="fcpool", bufs=1); fcpool = fc_cm.__enter__()
    w2p_cm = tc.tile_pool(name="w2pool", bufs=1); w2pool = w2p_cm.__enter__()
    w2sb2 = w2pool.tile([128, 4, 4096], BF16, name="w2sb2")
    nc.sync.dma_start(w2sb2[:], w2c[:].rearrange("(kt p) m -> p kt m", p=128))
    w3p_cm = tc.tile_pool(name="w3fpool", bufs=1); w3fpool = w3p_cm.__enter__()
    wt3 = w3fpool.tile([128, 32, NF3], BF16, name="w3t")
    nc.sync.dma_start(wt3[:], w3T[:].rearrange("(kt p) n -> p kt n", p=128))
    # ========================================================================
    # fc1: [64, 512] = feats_full.T @ w1c (+bias), relu
    # ========================================================================
    pf1 = psum.tile([64, 512], F32, name="pf1", tag="pf1", bufs=1)
    kt = 0
    for ct in range(2):
        for k in range(50):
            lhsT = featg2[:, ct, :, :, k]
            nc.tensor.matmul(pf1[:], lhsT, w1tiles[kt // 8][:, kt % 8, :],
                             start=(kt == 0), stop=False)
            kt += 1
    nc.tensor.matmul(pf1[:], ones_bf[:], b1csb[:], start=False, stop=True)
    f1 = fcpool.tile([64, 512], BF16, name="f1")
    nc.scalar.activation(f1[:], pf1[:], RELU)

    # transpose f1 -> f1T [128, 4, 64] bf16 via DMA transpose
    f1T = fcpool.tile([128, 4, 64], BF16, name="f1T")
    for t in range(4):
        nc.sync.dma_start_transpose(f1T[:, t, :], f1[:, 128 * t:128 * (t + 1)])

    # ========================================================================
    # fc2 partials (feature-major) -> AllReduce in two pipelined halves
    # ========================================================================
    part2 = fcpool.tile([128, 32, B], BF16, name="part2")
    f2pre = fcpool.tile([128, 32, B], BF16, name="f2pre")
    f2T = fcpool.tile([128, 32, B], BF16, name="f2T")
    ar_src_v = ar_src[:].rearrange("(m p) b -> p m b", p=128)
    ar_dst_v = ar_dst[:].rearrange("(m p) b -> p m b", p=128)
    for half in range(2):
        for mg in (2 * half, 2 * half + 1):  # 8 mt per psum bank
            p = ps("p_f2")
            for mi in range(8):
                mt = mg * 8 + mi
                for ktt in range(4):
                    nc.tensor.matmul(p[:, 64 * mi:64 * mi + B],
                                     w2sb2[:, ktt, 128 * mt:128 * (mt + 1)],
                                     f1T[:, ktt, :], start=(ktt == 0), stop=False)
                # gated bias: only core 0's ones0 is nonzero
                nc.tensor.matmul(p[:, 64 * mi:64 * mi + B],
                                 b2fsb[:, 128 * mt:128 * (mt + 1)], ones0[:],
                                 start=False, stop=True)
            if mg % 2 == 0:
                nc.scalar.activation(
                    part2[:, 8 * mg:8 * mg + 8, :].rearrange("p m b -> p (m b)"),
                    p[:], COPY)
            else:
                nc.vector.tensor_copy(
                    part2[:, 8 * mg:8 * mg + 8, :].rearrange("p m b -> p (m b)"),
                    p[:])
            smg = slice(8 * mg, 8 * mg + 8)
            nc.sync.dma_start(ar_src_v[:, smg, :], part2[:, smg, :])
        sl = slice(16 * half, 16 * half + 16)
        nc.gpsimd.collective_compute(
            "AllReduce", mybir.AluOpType.add,
            replica_groups=[list(range(N_CORES))],
            ins=[ar_src[2048 * half:2048 * half + 2048].opt()],
            outs=[ar_dst[2048 * half:2048 * half + 2048].opt()])
        nc.sync.dma_start(f2pre[:, sl, :], ar_dst_v[:, sl, :])
        nc.vector.tensor_scalar(f2T[:, sl, :].rearrange("p m b -> p (m b)"),
                                f2pre[:, sl, :].rearrange("p m b -> p (m b)"),
                                0.0, None, MAX)

    # ========================================================================
    # fc3 (tensor-parallel: NF3 output cols per core, host concatenates)
    # ========================================================================
    osb = fcpool.tile([64, NF3], F32, name="osb")
    p = ps("p_f3")
    for ktt in range(32):
        nc.tensor.matmul(p[:64, :NF3], f2T[:, ktt, :], wt3[:, ktt, :],
                         start=(ktt == 0), stop=False)
    nc.tensor.matmul(p[:64, :NF3], ones_bf[:], b3fsb[:],
                     start=False, stop=True)
    nc.scalar.activation(osb[:], p[:64, :NF3], COPY)
    nc.sync.dma_start(out[:], osb[:])

    w3p_cm.__exit__(None, None, None)
    w2p_cm.__exit__(None, None, None)
    fc_cm.__exit__(None, None, None)
    fg_cm.__exit__(None, None, None)
    w1_cm.__exit__(None, None, None)
    psum_cm.__exit__(None, None, None)
    const_cm.__exit__(None, None, None)
    tc_cm.__exit__(None, None, None)

    nc.compile()
    return nc


# ----------------------------------------------------------------------------
# host-side input preparation
# ----------------------------------------------------------------------------

def _prep_conv1(x):
    """x [B,3,224,224] fp32 -> per-core replicated tap strips [96,111,W1S] bf16."""
    Bb = x.shape[0]
    xpad = np.zeros((Bb, 3, 230, 230), np.float32)
    xpad[:, :, 3:227, 3:227] = x
    xph = np.empty((Bb, 2, 2, 3, 115, 115), np.float32)
    for p in range(2):
        for q in range(2):
            xph[:, p, q] = xpad[:, :, p:p + 229:2, q:q + 229:2]
    xph = xph.astype(ml_dtypes.bfloat16)
    reps = []
    for c in range(N_CORES):
        ph = xph[c * BL:(c + 1) * BL]  # [8, 2, 2, 3, 115, 115]
        rep = np.zeros((96, 111, W1S), ml_dtypes.bfloat16)
        k = 0
        for g01 in range(2):
            for a in range(4):
                blk = ph[:, :, :, :, a:a + 111, :]  # [8,2,2,3,111,115]
                v = np.zeros((2, 2, 3, 111, BL, 115), ml_dtypes.bfloat16)
                v[:, :, :, :, :, :115 - g01] = np.transpose(
                    blk[..., g01:], (1, 2, 3, 4, 0, 5))
                rep[k:k + 12, :, :920] = v.reshape(12, 111, BL * 115)
                k += 12
        reps.append(rep)
    return reps


def _prep_w1(w1):
    """w1 [64,3,7,7] -> w1g [2 groups, 96, 64] bf16 (zero-padded taps)."""
    w1g = np.zeros((2, 96, 64), np.float32)
    for g in range(2):
        k = 0
        for g01 in range(2):
            for a in range(4):
                for p in range(2):
                    for q in range(2):
                        di = 2 * a + p
                        dj = 2 * (g01 + 2 * g) + q
                        for c in range(3):
                            if di <= 6 and dj <= 6:
                                w1g[g, k] = w1[:, c, di, dj]
                            k += 1
    return w1g.astype(ml_dtypes.bfloat16)


def _spp_counts():
    cnt = np.empty(50, np.float32)
    for kk, (i0, i1, j0, j1) in enumerate(_spp_bins()):
        cnt[kk] = (i1 - i0) * (j1 - j0)
    return cnt


_CACHED = {}


def kernel(**inputs):
    if "nc" not in _CACHED:
        _CACHED["nc"] = build_program()
    nc = _CACHED["nc"]

    x = np.asarray(inputs["x"], np.float32)
    reps = _prep_conv1(x)
    w1gv = _prep_w1(np.asarray(inputs["w1"], np.float32))
    b1v = np.asarray(inputs["b1"], np.float32).reshape(64, 1)

    fc1_w = np.asarray(inputs["fc1_w"], np.float32)
    fc1_b = np.asarray(inputs["fc1_b"], np.float32)
    fc2_w = np.asarray(inputs["fc2_w"], np.float32)
    fc2_b = np.asarray(inputs["fc2_b"], np.float32)
    fc3_w = np.asarray(inputs["fc3_w"], np.float32)
    fc3_b = np.asarray(inputs["fc3_b"], np.float32)

    cnt = _spp_counts()
    w1s = fc1_w.reshape(4096, 256, 50) / cnt[None, None, :]
    # device feature d = (ct*50 + k)*128 + c128 -> channel ct*128+c128, bin k
    w1d = np.ascontiguousarray(
        w1s.reshape(4096, 2, 128, 50).transpose(1, 3, 2, 0))  # [2, 50, 128, 4096]

    w2cT = fc2_w.T  # [4096(in rows), 4096(out cols)] -> slice rows per core
    w3Tv = np.ascontiguousarray(fc3_w.T).astype(ml_dtypes.bfloat16)  # [4096, 1000]
    b3fv = fc3_b.reshape(1, 1000).astype(ml_dtypes.bfloat16)
    b2fv = fc2_b.reshape(1, 4096).astype(ml_dtypes.bfloat16)
    ones_one = np.ones((1, B), ml_dtypes.bfloat16)
    ones_zero = np.zeros((1, B), ml_dtypes.bfloat16)

    conv_w = {}
    for i in range(4):
        conv_w[f"w2_{i}"] = np.ascontiguousarray(
            np.asarray(inputs[f"w2_{i}"], np.float32).transpose(2, 3, 1, 0)
        ).astype(ml_dtypes.bfloat16)
        conv_w[f"b2_{i}"] = np.asarray(inputs[f"b2_{i}"], np.float32).reshape(128, 1)
    for i in range(6):
        conv_w[f"w3_{i}"] = np.ascontiguousarray(
            np.asarray(inputs[f"w3_{i}"], np.float32).transpose(2, 3, 1, 0)
        ).astype(ml_dtypes.bfloat16)
        conv_w[f"b3_{i}"] = np.ascontiguousarray(
            np.asarray(inputs[f"b3_{i}"], np.float32).reshape(2, 128).T)

    in_maps = []
    for c in range(N_CORES):
        sl = slice(512 * c, 512 * (c + 1))
        sl3 = slice(NF3 * c, NF3 * (c + 1))
        m = {
            "c1rep": reps[c],
            "w1g": w1gv,
            "b1": b1v,
            "w1c": np.ascontiguousarray(w1d[:, :, :, sl]).reshape(
                100, 128, 512).astype(ml_dtypes.bfloat16),
            "b1c": fc1_b[sl].reshape(1, 512).astype(ml_dtypes.bfloat16),
            "w2c": np.ascontiguousarray(w2cT[sl]).astype(ml_dtypes.bfloat16),
            "b2f": b2fv,
            "ones0": ones_one if c == 0 else ones_zero,
            "w3T": np.ascontiguousarray(w3Tv[:, sl3]),
            "b3f": np.ascontiguousarray(b3fv[:, sl3]),
        }
        m.update(conv_w)
        in_maps.append(m)

    res = run_bass_kernel_spmd(
        nc, in_maps, core_ids=list(range(N_CORES)),
        trace=bool(_CACHED.get("trace")), tmpdir=_CACHED.get("tmpdir"))
    _CACHED["last_result"] = res
    return np.concatenate(
        [np.asarray(res.results[c]["out"], np.float32) for c in range(N_CORES)],
        axis=1)
